# revision 31
# baseline (speedup 1.0000x reference)
"""Multi-head self-attention (8 heads, head_dim 64, n=4096, dim=256) on 8
Trainium2 NeuronCores.

Sharding: one attention head per core (tensor parallel on the heads axis of
to_qkv / to_out). Each core:
  A) computes the dual-layout projections qk = [q; k] and kq = [k; q]
     (each 128 x 4096 bf16, q/k on opposite partition halves) plus v in
     fp8e4 (pair layout for DoubleRow, with a ones column so the softmax
     denominator falls out of the PE matmul),
  B) streams the 4096x4096 attention for its head in j-tile PAIRS:
     sim = k^T q for two j-tiles lands in one 2-bank PSUM tile; the
     shifted exp (logits - 2 keeps exp < 240 = fp8e4 max) is produced in
     fp8 by one of TWO engines round-robin — the scalar engine (exact exp,
     fp8 out) or the vector engine via a Schraudolph-style uint8 bit trick
     (affine + saturating u8 convert, bitcast as fp8e4) — so exp never
     gates the PE and the PE stays continuously busy (full 2.4 GHz
     p-state).  The attention-weighted value sum runs as one fp8 DoubleRow
     matmul per pair (2 j-tiles per ~256-cycle pass) accumulated in PSUM.
  C) normalizes by the fused row sum (copy + DVE reciprocal + a K=1 fp16
     matmul that replicates 1/s across partitions, written back into the
     just-freed PSUM bank), then an AllToAll gives every core the full
     512-channel hidden state for its own 512-token slice; the final
     projection + bias runs per core on that slice.
The host only reshapes/slices/casts inputs per core and concatenates the 8
disjoint token slices of the output.
"""

import os
import sys
from contextlib import ExitStack

for _p in ("/opt/trn_rl_repo",):
    if os.path.isdir(_p) and _p not in sys.path:
        sys.path.append(_p)

import ml_dtypes
import numpy as np

import concourse.bass as bass
import concourse.mybir as mybir
import concourse.tile as tile
from concourse import bacc
from concourse.bass_utils import run_bass_kernel_spmd

HEADS = 8
HD = 64           # head dim
DIM = 256         # model dim
N = 4096          # tokens (64*64)
HID = HEADS * HD  # 512
NB = 8            # token blocks
BLK = N // NB     # 512
NJ = N // 128     # 32 j-tiles of 128
NG = NJ // 2      # 16 j-tile pairs per i-block
N_CORES = 8

F32 = mybir.dt.float32
F16 = mybir.dt.float16
BF16 = mybir.dt.bfloat16
FP8 = mybir.dt.float8e4
U8 = mybir.dt.uint8
EXP = mybir.ActivationFunctionType.Exp
DR = mybir.MatmulPerfMode.DoubleRow

LOG2E = 1.4426950408889634
C_SHIFT = 2.0                                    # logit shift: exp < 240
SCHR_SCALE = float(8 * LOG2E)                    # fp8e4 Schraudolph slope
SCHR_BIAS = float(56.0 - C_SHIFT * 8 * LOG2E - 0.5)  # bias 7<<3, magic -0.5

# exp-engine round robin per pair: A=scalar(exact fp8), D=vector (schr-u8)
ASSIGN_MAIN = "ADAADAADAADAADAD"   # 10 A, 6 D  (blocks 1..7)
ASSIGN_BLK0 = "ADADADADADADADAA"   # 9 A, 7 D   (stage A loads the engines)
if os.environ.get("KERNEL_ALL_SCHR", "0") == "1":
    ASSIGN_MAIN = ASSIGN_BLK0 = "D" * 16
if os.environ.get("KERNEL_ALL_ACT", "0") == "1":
    ASSIGN_MAIN = ASSIGN_BLK0 = "A" * 16


def build_program():
    nc = bacc.Bacc("TRN2", target_bir_lowering=False, debug=False,
                   num_devices=N_CORES)
    x_d = nc.declare_dram_parameter("x", [DIM, N], BF16, isOutput=False)
    # columns: [wq|wk | wk|wq | wv] (wq pre-scaled by head_dim**-0.5)
    wqkvT_d = nc.declare_dram_parameter("wqkvT", [DIM, 320], BF16,
                                        isOutput=False)
    woT_d = nc.declare_dram_parameter("woT", [HID, DIM], BF16, isOutput=False)
    b_d = nc.declare_dram_parameter("bout", [DIM], F32, isOutput=False)
    y_d = nc.declare_dram_parameter("y", [DIM, BLK], F32, isOutput=True)

    with tile.TileContext(nc) as tc, ExitStack() as ctx:
        const = ctx.enter_context(tc.tile_pool(name="const", bufs=1))
        sbA = ctx.enter_context(tc.tile_pool(name="sbA", bufs=1))
        pexp = ctx.enter_context(tc.tile_pool(name="pexp", bufs=6))
        psml = ctx.enter_context(tc.tile_pool(name="psml", bufs=3))
        dram = ctx.enter_context(tc.tile_pool(name="dram", bufs=1,
                                              space="DRAM"))
        psO = ctx.enter_context(tc.tile_pool(name="psO", bufs=2,
                                             space="PSUM"))

        # ---- constants / persistent SBUF ----
        wqkvT_sb = const.tile([128, 2, 320], BF16)
        nc.sync.dma_start(wqkvT_sb[:],
                          wqkvT_d.rearrange("(c p) m -> p c m", p=128))
        # final-projection weights loaded up front (off the critical tail)
        woT_sb = const.tile([128, 4, DIM], BF16)
        nc.sync.dma_start(woT_sb[:],
                          woT_d.rearrange("(c p) m -> p c m", p=128))
        b_sb = const.tile([128, 2], F32)
        nc.sync.dma_start(b_sb[:], b_d.rearrange("(m p) -> p m", p=128))

        nbias_sb = const.tile([128, 1], F32)
        nc.vector.memset(nbias_sb[:], -C_SHIFT)
        ones16_sb = const.tile([128, HD], F16)
        nc.vector.memset(ones16_sb[:], 1.0)

        x_sb = sbA.tile([128, 2, N], BF16)
        qk_sb = sbA.tile([128, N], BF16)   # partitions 0:64 = q, 64:128 = k
        kq_sb = sbA.tile([128, N], BF16)   # partitions 0:64 = k, 64:128 = q
        # v fp8 pair tiles: [p, pair, ktile, dim]; col 64 = ones so the
        # denominator accumulates in psum row 64; cols padded to 80 for the
        # DoubleRow ldweights 16B stride rule.
        vv_sb = sbA.tile([128, NG, 2, 80], FP8)
        nc.vector.memset(vv_sb[:], 0.0)
        nc.vector.memset(vv_sb[:, :, :, 64:65], 1.0)
        rhs_sb = sbA.tile([128, 4, BLK], BF16)

        a2a_in = dram.tile([NB, HD, BLK], BF16)
        a2a_out = dram.tile([NB, HD, BLK], BF16)
        warm_in = dram.tile([128, 4], F32)
        warm_out = dram.tile([128, 4], F32)
        warmz_sb = const.tile([128, 4], F32)
        nc.vector.memset(warmz_sb[:], 0.0)
        nc.sync.dma_start(warm_in[:], warmz_sb[:])
        # tiny warm-up collective: absorbs CC init cost under the preamble
        nc.gpsimd.collective_compute(
            "AllReduce", mybir.AluOpType.add,
            replica_groups=[list(range(N_CORES))],
            ins=[warm_in.opt()], outs=[warm_out.opt()])
        # same-shape dummy AllToAll: pre-builds the collective's rings and
        # descriptors so the real one at the tail starts faster
        warm2_in = dram.tile([NB, HD, BLK], BF16)
        warm2_out = dram.tile([NB, HD, BLK], BF16)
        nc.gpsimd.dma_start(warm2_in[0:1, 0:1, 0:4],
                            warmz_sb[0:1, 0:4])
        nc.gpsimd.collective_compute(
            "AllToAll", mybir.AluOpType.bypass,
            replica_groups=[list(range(N_CORES))],
            ins=[warm2_in.opt()], outs=[warm2_out.opt()])

        pending = []  # [countdown, fn] emitted in order once countdown <= 0

        def schedule(fn, delay):
            pending.append([delay, fn])

        def tick():
            for it in pending:
                it[0] -= 1
            for it in [it for it in pending if it[0] <= 0]:
                pending.remove(it)
                it[1]()

        def drain():
            while pending:
                pending.pop(0)[1]()

        def emit_stage_a(b, psA):
            bs = slice(b * BLK, (b + 1) * BLK)
            for c in range(2):
                nc.sync.dma_start(x_sb[:, c, bs],
                                  x_d[c * 128:(c + 1) * 128, bs])
            ps_qk = psA.tile([128, BLK], F32, tag="pa", name=f"psqk_{b}")
            for c in range(2):
                nc.tensor.matmul(ps_qk[:], wqkvT_sb[:, c, 0:128],
                                 x_sb[:, c, bs],
                                 start=(c == 0), stop=(c == 1))
            nc.vector.tensor_copy(qk_sb[:, bs], ps_qk[:])
            ps_kq = psA.tile([128, BLK], F32, tag="pa", name=f"pskq_{b}")
            for c in range(2):
                nc.tensor.matmul(ps_kq[:], wqkvT_sb[:, c, 128:256],
                                 x_sb[:, c, bs],
                                 start=(c == 0), stop=(c == 1))
            nc.vector.tensor_copy(kq_sb[:, bs], ps_kq[:])
            for t in range(4):
                nt = b * 4 + t
                g, kt = nt // 2, nt % 2
                ps_v = psA.tile([128, HD], F32, tag="pa", name=f"psv_{nt}")
                for c in range(2):
                    nc.tensor.matmul(
                        ps_v[:],
                        x_sb[:, c, nt * 128:(nt + 1) * 128],
                        wqkvT_sb[:, c, 256:320],
                        start=(c == 0), stop=(c == 1))
                nc.vector.tensor_copy(vv_sb[:, g, kt, 0:64], ps_v[:])

        def emit_pair(i, g, ps_out, psB, assign):
            isl = slice(i * BLK, (i + 1) * BLK)
            j0, j1 = 2 * g, 2 * g + 1
            psg = psB.tile([128, 2, BLK], F32, tag="psg",
                           name=f"psg_{i}_{g}")
            nc.tensor.matmul(psg[:, 0, :],
                             kq_sb[0:64, j0 * 128:(j0 + 1) * 128],
                             qk_sb[0:64, isl],
                             start=True, stop=True, tile_position=(0, 0))
            nc.tensor.matmul(psg[:, 1, :],
                             qk_sb[64:128, j1 * 128:(j1 + 1) * 128],
                             kq_sb[64:128, isl],
                             start=True, stop=True, tile_position=(64, 0))
            pe = pexp.tile([128, 2, BLK], FP8, tag="pe", name=f"pe_{i}_{g}")
            if assign[g % NG] == 'A':
                nc.scalar.activation(pe[:], psg[:], EXP, bias=nbias_sb[:])
            else:
                # saturating affine+u8 convert on DVE (hardware clamps
                # negative f32->u8 to 0; only CoreSim models a wrap)
                nc.vector.tensor_scalar(pe[:].bitcast(U8), psg[:],
                                        SCHR_SCALE, SCHR_BIAS,
                                        mybir.AluOpType.mult,
                                        mybir.AluOpType.add)

            def mk_outp():
                nc.tensor.matmul(ps_out[0:65, :], vv_sb[:, g, :, 0:65],
                                 pe[:], start=(g == 0), stop=(g == NG - 1),
                                 perf_mode=DR)
            schedule(mk_outp, 2)
            tick()

        def emit_norm(i, ps_out):
            oall = psml.tile([128, BLK], F32, tag="oall", name=f"oall_{i}")
            r_sb = psml.tile([128, BLK], F32, tag="rsb", name=f"rsb_{i}")
            r16_sb = psml.tile([128, BLK], F16, tag="r16", name=f"r16_{i}")

            def mk_norm_a():
                nc.vector.tensor_copy(oall[0:65, :], ps_out[0:65, :])
                nc.vector.reciprocal(r_sb[64:65, :], oall[64:65, :])
                nc.vector.tensor_copy(r16_sb[64:65, :], r_sb[64:65, :])
            schedule(mk_norm_a, 2)

            def mk_norm_b():
                outn = psml.tile([HD, BLK], BF16, tag="outn",
                                 name=f"outn_{i}")
                # ps_out rows 0:64 are free once oall holds the copy: write
                # the fp16 K=1 broadcast of 1/s right back into them
                nc.tensor.matmul(ps_out[0:HD, :], ones16_sb[64:65, 0:HD],
                                 r16_sb[64:65, :], start=True, stop=True)
                nc.vector.tensor_mul(outn[:], oall[0:HD, :],
                                     ps_out[0:HD, :])
                nc.sync.dma_start(a2a_in[i], outn[:])
            schedule(mk_norm_b, 5)

        # ---- stage A interleaved with i-block 0 ----
        ps_out0 = psO.tile([128, BLK], F32, tag="psout", name="psout_0")
        with tc.tile_pool(name="psA", bufs=2, space="PSUM") as psA_pool, \
                tc.tile_pool(name="psB2", bufs=2, space="PSUM") as psB2:
            for b in range(NB):
                emit_stage_a(b, psA_pool)
                emit_pair(0, 2 * b, ps_out0, psB2, ASSIGN_BLK0)
                emit_pair(0, 2 * b + 1, ps_out0, psB2, ASSIGN_BLK0)
            emit_norm(0, ps_out0)

        # ---- i-blocks 1..7 ----
        with tc.tile_pool(name="psB3", bufs=3, space="PSUM") as psB3:
            for i in range(1, NB):
                ps_out = psO.tile([128, BLK], F32, tag="psout",
                                  name=f"psout_{i}")
                for g in range(NG):
                    emit_pair(i, g, ps_out, psB3, ASSIGN_MAIN)
                emit_norm(i, ps_out)
            drain()

            # ---- stage C: AllToAll over token blocks + output projection --
            nc.gpsimd.collective_compute(
                "AllToAll", mybir.AluOpType.bypass,
                replica_groups=[list(range(N_CORES))],
                ins=[a2a_in.opt()], outs=[a2a_out.opt()])

            a2a_r = a2a_out.rearrange("(c a) d t -> (a d) c t", c=4, a=2)
            ps_yt = psB3.tile([128, 2, BLK], F32, tag="psg", name="psy")
            for c in range(4):
                nc.gpsimd.dma_start(rhs_sb[:, c, :], a2a_r[:, c, :])
                for m in range(2):
                    nc.tensor.matmul(ps_yt[:, m, :],
                                     woT_sb[:, c, m * 128:(m + 1) * 128],
                                     rhs_sb[:, c, :],
                                     start=(c == 0), stop=(c == 3))
            for m in range(2):
                y_sb = psml.tile([128, BLK], F32, tag="ysb", name=f"ysb_{m}")
                nc.vector.tensor_scalar_add(y_sb[:], ps_yt[:, m, :],
                                            b_sb[:, m:m + 1])
                nc.sync.dma_start(y_d[m * 128:(m + 1) * 128, :], y_sb[:])

    nc.compile()
    return nc


def _make_in_maps(x, w_qkv, w_out, b_out):
    x2 = np.ascontiguousarray(
        np.asarray(x, np.float32).reshape(DIM, N)).astype(ml_dtypes.bfloat16)
    w_qkv = np.asarray(w_qkv, np.float32)
    scale = HD ** -0.5
    woT = np.ascontiguousarray(np.asarray(w_out, np.float32).T).astype(
        ml_dtypes.bfloat16)
    b = np.ascontiguousarray(np.asarray(b_out, np.float32).reshape(DIM))
    in_maps = []
    for h in range(N_CORES):
        wq = w_qkv[h * HD:(h + 1) * HD] * scale
        wk = w_qkv[HID + h * HD:HID + (h + 1) * HD]
        wv = w_qkv[2 * HID + h * HD:2 * HID + (h + 1) * HD]
        wqkvT = np.ascontiguousarray(
            np.concatenate([wq.T, wk.T, wk.T, wq.T, wv.T], axis=1),
            np.float32).astype(ml_dtypes.bfloat16)
        in_maps.append({"x": x2, "wqkvT": wqkvT, "woT": woT, "bout": b})
    return in_maps


def _assemble(results):
    y = np.concatenate([results[h]["y"] for h in range(N_CORES)], axis=1)
    return np.ascontiguousarray(y.reshape(1, DIM, 64, 64).astype(np.float32))


def kernel(x, w_qkv, w_out, b_out):
    nc = build_program()
    in_maps = _make_in_maps(x, w_qkv, w_out, b_out)
    res = run_bass_kernel_spmd(nc, in_maps, list(range(N_CORES)))
    return _assemble(res.results)


def run_traced(x, w_qkv, w_out, b_out, trace_cores=None):
    """Test-harness entry: also returns BassKernelResults with exec_time_ns."""
    nc = build_program()
    in_maps = _make_in_maps(x, w_qkv, w_out, b_out)
    res = run_bass_kernel_spmd(nc, in_maps, list(range(N_CORES)), trace=True,
                               trace_cores=trace_cores)
    return _assemble(res.results), res


# revision 32
# speedup vs baseline: 1.0412x; 1.0412x over previous
"""Multi-head self-attention (8 heads, head_dim 64, n=4096, dim=256) on 8
Trainium2 NeuronCores.

Sharding: one attention head per core (tensor parallel on the heads axis of
to_qkv / to_out). Each core:
  A) computes the dual-layout projections qk = [q; k] and kq = [k; q]
     (each 128 x 4096 bf16, q/k on opposite partition halves) plus v in
     fp8e4 (pair layout for DoubleRow, with a ones column so the softmax
     denominator falls out of the PE matmul),
  B) streams the 4096x4096 attention for its head in j-tile PAIRS:
     sim = k^T q for two j-tiles lands in one 2-bank PSUM tile; the
     shifted exp (logits - 2 keeps exp < 240 = fp8e4 max) is produced in
     fp8 by one of TWO engines round-robin — the scalar engine (exact exp,
     fp8 out) or the vector engine via a Schraudolph-style uint8 bit trick
     (affine + saturating u8 convert, bitcast as fp8e4) — so exp never
     gates the PE and the PE stays continuously busy (full 2.4 GHz
     p-state).  The attention-weighted value sum runs as one fp8 DoubleRow
     matmul per pair (2 j-tiles per ~256-cycle pass) accumulated in PSUM.
  C) normalizes by the fused row sum (copy + DVE reciprocal + a K=1 fp16
     matmul that replicates 1/s across partitions, written back into the
     just-freed PSUM bank), then an AllToAll gives every core the full
     512-channel hidden state for its own 512-token slice; the final
     projection + bias runs per core on that slice.
The host only reshapes/slices/casts inputs per core and concatenates the 8
disjoint token slices of the output.
"""

import os
import sys
from contextlib import ExitStack

for _p in ("/opt/trn_rl_repo",):
    if os.path.isdir(_p) and _p not in sys.path:
        sys.path.append(_p)

import ml_dtypes
import numpy as np

import concourse.bass as bass
import concourse.mybir as mybir
import concourse.tile as tile
from concourse import bacc
from concourse.bass_utils import run_bass_kernel_spmd

HEADS = 8
HD = 64           # head dim
DIM = 256         # model dim
N = 4096          # tokens (64*64)
HID = HEADS * HD  # 512
NB = 8            # token blocks
BLK = N // NB     # 512
NJ = N // 128     # 32 j-tiles of 128
NG = NJ // 2      # 16 j-tile pairs per i-block
N_CORES = 8

F32 = mybir.dt.float32
F16 = mybir.dt.float16
BF16 = mybir.dt.bfloat16
FP8 = mybir.dt.float8e4
U8 = mybir.dt.uint8
EXP = mybir.ActivationFunctionType.Exp
DR = mybir.MatmulPerfMode.DoubleRow

LOG2E = 1.4426950408889634
C_SHIFT = 2.0                                    # logit shift: exp < 240
SCHR_SCALE = float(8 * LOG2E)                    # fp8e4 Schraudolph slope
SCHR_BIAS = float(56.0 - C_SHIFT * 8 * LOG2E - 0.5)  # bias 7<<3, magic -0.5

# exp-engine round robin per pair: A=scalar(exact fp8), D=vector (schr-u8)
ASSIGN_MAIN = "ADAADAADAADAADAD"   # 10 A, 6 D  (blocks 1..7)
ASSIGN_BLK0 = "ADADADADADADADAA"   # 9 A, 7 D   (stage A loads the engines)
if os.environ.get("KERNEL_ALL_SCHR", "0") == "1":
    ASSIGN_MAIN = ASSIGN_BLK0 = "D" * 16
if os.environ.get("KERNEL_ALL_ACT", "0") == "1":
    ASSIGN_MAIN = ASSIGN_BLK0 = "A" * 16


def build_program():
    nc = bacc.Bacc("TRN2", target_bir_lowering=False, debug=False,
                   num_devices=N_CORES)
    x_d = nc.declare_dram_parameter("x", [DIM, N], BF16, isOutput=False)
    # columns: [wq|wk | wk|wq | wv] (wq pre-scaled by head_dim**-0.5)
    wqkvT_d = nc.declare_dram_parameter("wqkvT", [DIM, 320], BF16,
                                        isOutput=False)
    woT_d = nc.declare_dram_parameter("woT", [HID, DIM], BF16, isOutput=False)
    b_d = nc.declare_dram_parameter("bout", [DIM], F32, isOutput=False)
    y_d = nc.declare_dram_parameter("y", [DIM, BLK], F32, isOutput=True)

    with tile.TileContext(nc) as tc, ExitStack() as ctx:
        const = ctx.enter_context(tc.tile_pool(name="const", bufs=1))
        sbA = ctx.enter_context(tc.tile_pool(name="sbA", bufs=1))
        pexp = ctx.enter_context(tc.tile_pool(name="pexp", bufs=6))
        psml = ctx.enter_context(tc.tile_pool(name="psml", bufs=3))
        dram = ctx.enter_context(tc.tile_pool(name="dram", bufs=1,
                                              space="DRAM"))
        psO = ctx.enter_context(tc.tile_pool(name="psO", bufs=2,
                                             space="PSUM"))

        # ---- constants / persistent SBUF ----
        wqkvT_sb = const.tile([128, 2, 320], BF16)
        nc.sync.dma_start(wqkvT_sb[:],
                          wqkvT_d.rearrange("(c p) m -> p c m", p=128))
        # final-projection weights loaded up front (off the critical tail)
        woT_sb = const.tile([128, 4, DIM], BF16)
        nc.sync.dma_start(woT_sb[:],
                          woT_d.rearrange("(c p) m -> p c m", p=128))
        b_sb = const.tile([128, 2], F32)
        nc.sync.dma_start(b_sb[:], b_d.rearrange("(m p) -> p m", p=128))

        nbias_sb = const.tile([128, 1], F32)
        nc.vector.memset(nbias_sb[:], -C_SHIFT)
        ones16_sb = const.tile([128, HD], F16)
        nc.vector.memset(ones16_sb[:], 1.0)

        x_sb = sbA.tile([128, 2, N], BF16)
        qk_sb = sbA.tile([128, N], BF16)   # partitions 0:64 = q, 64:128 = k
        kq_sb = sbA.tile([128, N], BF16)   # partitions 0:64 = k, 64:128 = q
        # v fp8 pair tiles: [p, pair, ktile, dim]; col 64 = ones so the
        # denominator accumulates in psum row 64; cols padded to 80 for the
        # DoubleRow ldweights 16B stride rule.
        vv_sb = sbA.tile([128, NG, 2, 80], FP8)
        nc.vector.memset(vv_sb[:], 0.0)
        nc.vector.memset(vv_sb[:, :, :, 64:65], 1.0)
        rhs_sb = sbA.tile([128, 4, BLK], BF16)

        a2a_in = dram.tile([NB, HD, BLK], BF16)
        a2a_out = dram.tile([NB, HD, BLK], BF16)
        warm_in = dram.tile([128, 4], F32)
        warm_out = dram.tile([128, 4], F32)
        warmz_sb = const.tile([128, 4], F32)
        nc.vector.memset(warmz_sb[:], 0.0)
        nc.sync.dma_start(warm_in[:], warmz_sb[:])
        # tiny warm-up collective: absorbs CC init cost under the preamble
        nc.gpsimd.collective_compute(
            "AllReduce", mybir.AluOpType.add,
            replica_groups=[list(range(N_CORES))],
            ins=[warm_in.opt()], outs=[warm_out.opt()])

        pending = []  # [countdown, fn] emitted in order once countdown <= 0

        def schedule(fn, delay):
            pending.append([delay, fn])

        def tick():
            for it in pending:
                it[0] -= 1
            for it in [it for it in pending if it[0] <= 0]:
                pending.remove(it)
                it[1]()

        def drain():
            while pending:
                pending.pop(0)[1]()

        def emit_stage_a(b, psA):
            bs = slice(b * BLK, (b + 1) * BLK)
            for c in range(2):
                nc.sync.dma_start(x_sb[:, c, bs],
                                  x_d[c * 128:(c + 1) * 128, bs])
            ps_qk = psA.tile([128, BLK], F32, tag="pa", name=f"psqk_{b}")
            for c in range(2):
                nc.tensor.matmul(ps_qk[:], wqkvT_sb[:, c, 0:128],
                                 x_sb[:, c, bs],
                                 start=(c == 0), stop=(c == 1))
            nc.vector.tensor_copy(qk_sb[:, bs], ps_qk[:])
            ps_kq = psA.tile([128, BLK], F32, tag="pa", name=f"pskq_{b}")
            for c in range(2):
                nc.tensor.matmul(ps_kq[:], wqkvT_sb[:, c, 128:256],
                                 x_sb[:, c, bs],
                                 start=(c == 0), stop=(c == 1))
            nc.vector.tensor_copy(kq_sb[:, bs], ps_kq[:])
            for t in range(4):
                nt = b * 4 + t
                g, kt = nt // 2, nt % 2
                ps_v = psA.tile([128, HD], F32, tag="pa", name=f"psv_{nt}")
                for c in range(2):
                    nc.tensor.matmul(
                        ps_v[:],
                        x_sb[:, c, nt * 128:(nt + 1) * 128],
                        wqkvT_sb[:, c, 256:320],
                        start=(c == 0), stop=(c == 1))
                nc.vector.tensor_copy(vv_sb[:, g, kt, 0:64], ps_v[:])

        def emit_pair(i, g, ps_out, psB, assign):
            isl = slice(i * BLK, (i + 1) * BLK)
            j0, j1 = 2 * g, 2 * g + 1
            psg = psB.tile([128, 2, BLK], F32, tag="psg",
                           name=f"psg_{i}_{g}")
            nc.tensor.matmul(psg[:, 0, :],
                             kq_sb[0:64, j0 * 128:(j0 + 1) * 128],
                             qk_sb[0:64, isl],
                             start=True, stop=True, tile_position=(0, 0))
            nc.tensor.matmul(psg[:, 1, :],
                             qk_sb[64:128, j1 * 128:(j1 + 1) * 128],
                             kq_sb[64:128, isl],
                             start=True, stop=True, tile_position=(64, 0))
            pe = pexp.tile([128, 2, BLK], FP8, tag="pe", name=f"pe_{i}_{g}")
            if assign[g % NG] == 'A':
                nc.scalar.activation(pe[:], psg[:], EXP, bias=nbias_sb[:])
            else:
                # saturating affine+u8 convert on DVE (hardware clamps
                # negative f32->u8 to 0; only CoreSim models a wrap)
                nc.vector.tensor_scalar(pe[:].bitcast(U8), psg[:],
                                        SCHR_SCALE, SCHR_BIAS,
                                        mybir.AluOpType.mult,
                                        mybir.AluOpType.add)

            def mk_outp():
                nc.tensor.matmul(ps_out[0:65, :], vv_sb[:, g, :, 0:65],
                                 pe[:], start=(g == 0), stop=(g == NG - 1),
                                 perf_mode=DR)
            schedule(mk_outp, 2)
            tick()

        def emit_norm(i, ps_out):
            oall = psml.tile([128, BLK], F32, tag="oall", name=f"oall_{i}")
            r_sb = psml.tile([128, BLK], F32, tag="rsb", name=f"rsb_{i}")
            r16_sb = psml.tile([128, BLK], F16, tag="r16", name=f"r16_{i}")

            def mk_norm_a():
                nc.vector.tensor_copy(oall[0:65, :], ps_out[0:65, :])
                nc.vector.reciprocal(r_sb[64:65, :], oall[64:65, :])
                nc.vector.tensor_copy(r16_sb[64:65, :], r_sb[64:65, :])
            schedule(mk_norm_a, 2)

            def mk_norm_b():
                outn = psml.tile([HD, BLK], BF16, tag="outn",
                                 name=f"outn_{i}")
                # ps_out rows 0:64 are free once oall holds the copy: write
                # the fp16 K=1 broadcast of 1/s right back into them
                nc.tensor.matmul(ps_out[0:HD, :], ones16_sb[64:65, 0:HD],
                                 r16_sb[64:65, :], start=True, stop=True)
                nc.vector.tensor_mul(outn[:], oall[0:HD, :],
                                     ps_out[0:HD, :])
                nc.sync.dma_start(a2a_in[i], outn[:])
            schedule(mk_norm_b, 5)

        # ---- stage A interleaved with i-block 0 ----
        ps_out0 = psO.tile([128, BLK], F32, tag="psout", name="psout_0")
        with tc.tile_pool(name="psA", bufs=2, space="PSUM") as psA_pool, \
                tc.tile_pool(name="psB2", bufs=2, space="PSUM") as psB2:
            for b in range(NB):
                emit_stage_a(b, psA_pool)
                emit_pair(0, 2 * b, ps_out0, psB2, ASSIGN_BLK0)
                emit_pair(0, 2 * b + 1, ps_out0, psB2, ASSIGN_BLK0)
            emit_norm(0, ps_out0)

        # ---- i-blocks 1..7 ----
        with tc.tile_pool(name="psB3", bufs=3, space="PSUM") as psB3:
            for i in range(1, NB):
                ps_out = psO.tile([128, BLK], F32, tag="psout",
                                  name=f"psout_{i}")
                for g in range(NG):
                    emit_pair(i, g, ps_out, psB3, ASSIGN_MAIN)
                emit_norm(i, ps_out)
            drain()

            # ---- stage C: AllToAll over token blocks + output projection --
            nc.gpsimd.collective_compute(
                "AllToAll", mybir.AluOpType.bypass,
                replica_groups=[list(range(N_CORES))],
                ins=[a2a_in.opt()], outs=[a2a_out.opt()])

            a2a_r = a2a_out.rearrange("(c a) d t -> (a d) c t", c=4, a=2)
            ps_yt = psB3.tile([128, 2, BLK], F32, tag="psg", name="psy")
            for c in range(4):
                nc.gpsimd.dma_start(rhs_sb[:, c, :], a2a_r[:, c, :])
                for m in range(2):
                    nc.tensor.matmul(ps_yt[:, m, :],
                                     woT_sb[:, c, m * 128:(m + 1) * 128],
                                     rhs_sb[:, c, :],
                                     start=(c == 0), stop=(c == 3))
            for m in range(2):
                y_sb = psml.tile([128, BLK], F32, tag="ysb", name=f"ysb_{m}")
                nc.vector.tensor_scalar_add(y_sb[:], ps_yt[:, m, :],
                                            b_sb[:, m:m + 1])
                nc.sync.dma_start(y_d[m * 128:(m + 1) * 128, :], y_sb[:])

    nc.compile()
    return nc


def _make_in_maps(x, w_qkv, w_out, b_out):
    x2 = np.ascontiguousarray(
        np.asarray(x, np.float32).reshape(DIM, N)).astype(ml_dtypes.bfloat16)
    w_qkv = np.asarray(w_qkv, np.float32)
    scale = HD ** -0.5
    woT = np.ascontiguousarray(np.asarray(w_out, np.float32).T).astype(
        ml_dtypes.bfloat16)
    b = np.ascontiguousarray(np.asarray(b_out, np.float32).reshape(DIM))
    in_maps = []
    for h in range(N_CORES):
        wq = w_qkv[h * HD:(h + 1) * HD] * scale
        wk = w_qkv[HID + h * HD:HID + (h + 1) * HD]
        wv = w_qkv[2 * HID + h * HD:2 * HID + (h + 1) * HD]
        wqkvT = np.ascontiguousarray(
            np.concatenate([wq.T, wk.T, wk.T, wq.T, wv.T], axis=1),
            np.float32).astype(ml_dtypes.bfloat16)
        in_maps.append({"x": x2, "wqkvT": wqkvT, "woT": woT, "bout": b})
    return in_maps


def _assemble(results):
    y = np.concatenate([results[h]["y"] for h in range(N_CORES)], axis=1)
    return np.ascontiguousarray(y.reshape(1, DIM, 64, 64).astype(np.float32))


def kernel(x, w_qkv, w_out, b_out):
    nc = build_program()
    in_maps = _make_in_maps(x, w_qkv, w_out, b_out)
    res = run_bass_kernel_spmd(nc, in_maps, list(range(N_CORES)))
    return _assemble(res.results)


def run_traced(x, w_qkv, w_out, b_out, trace_cores=None):
    """Test-harness entry: also returns BassKernelResults with exec_time_ns."""
    nc = build_program()
    in_maps = _make_in_maps(x, w_qkv, w_out, b_out)
    res = run_bass_kernel_spmd(nc, in_maps, list(range(N_CORES)), trace=True,
                               trace_cores=trace_cores)
    return _assemble(res.results), res


# revision 33
# speedup vs baseline: 1.1167x; 1.0725x over previous
"""Multi-head self-attention (8 heads, head_dim 64, n=4096, dim=256) on 8
Trainium2 NeuronCores.

Sharding: one attention head per core (tensor parallel on the heads axis of
to_qkv / to_out). Each core:
  A) computes the dual-layout projections qk = [q; k] and kq = [k; q]
     (each 128 x 4096 bf16, q/k on opposite partition halves) plus v in
     fp8e4 (pair layout for DoubleRow, with a ones column so the softmax
     denominator falls out of the PE matmul),
  B) streams the 4096x4096 attention for its head in j-tile PAIRS:
     sim = k^T q for two j-tiles lands in one 2-bank PSUM tile; the
     shifted exp (logits - 2 keeps exp < 240 = fp8e4 max) is produced in
     fp8 by one of TWO engines round-robin — the scalar engine (exact exp,
     fp8 out) or the vector engine via a Schraudolph-style uint8 bit trick
     (affine + saturating u8 convert, bitcast as fp8e4) — so exp never
     gates the PE and the PE stays continuously busy (full 2.4 GHz
     p-state).  The attention-weighted value sum runs as one fp8 DoubleRow
     matmul per pair (2 j-tiles per ~256-cycle pass) accumulated in PSUM.
  C) normalizes by the fused row sum (copy + DVE reciprocal + a K=1 fp16
     matmul that replicates 1/s across partitions, written back into the
     just-freed PSUM bank), then an AllToAll gives every core the full
     512-channel hidden state for its own 512-token slice; the final
     projection + bias runs per core on that slice.
The host only reshapes/slices/casts inputs per core and concatenates the 8
disjoint token slices of the output.
"""

import os
import sys
from contextlib import ExitStack

for _p in ("/opt/trn_rl_repo",):
    if os.path.isdir(_p) and _p not in sys.path:
        sys.path.append(_p)

import ml_dtypes
import numpy as np

import concourse.bass as bass
import concourse.mybir as mybir
import concourse.tile as tile
from concourse import bacc
from concourse.bass_utils import run_bass_kernel_spmd

HEADS = 8
HD = 64           # head dim
DIM = 256         # model dim
N = 4096          # tokens (64*64)
HID = HEADS * HD  # 512
NB = 8            # token blocks
BLK = N // NB     # 512
NJ = N // 128     # 32 j-tiles of 128
NG = NJ // 2      # 16 j-tile pairs per i-block
N_CORES = 8

F32 = mybir.dt.float32
F16 = mybir.dt.float16
BF16 = mybir.dt.bfloat16
FP8 = mybir.dt.float8e4
U8 = mybir.dt.uint8
EXP = mybir.ActivationFunctionType.Exp
DR = mybir.MatmulPerfMode.DoubleRow

LOG2E = 1.4426950408889634
C_SHIFT = 2.0                                    # logit shift: exp < 240
SCHR_SCALE = float(8 * LOG2E)                    # fp8e4 Schraudolph slope
SCHR_BIAS = float(56.0 - C_SHIFT * 8 * LOG2E - 0.5)  # bias 7<<3, magic -0.5

# exp-engine round robin per pair: A=scalar(exact fp8), D=vector (schr-u8)
ASSIGN_MAIN = "AADAADAADAADAADA"   # 11 A, 5 D  (blocks 1..7)
ASSIGN_BLK0 = "ADADADADADADADAA"   # 9 A, 7 D   (stage A loads the engines)
if os.environ.get("KERNEL_ALL_SCHR", "0") == "1":
    ASSIGN_MAIN = ASSIGN_BLK0 = "D" * 16
if os.environ.get("KERNEL_ALL_ACT", "0") == "1":
    ASSIGN_MAIN = ASSIGN_BLK0 = "A" * 16


def build_program():
    nc = bacc.Bacc("TRN2", target_bir_lowering=False, debug=False,
                   num_devices=N_CORES)
    x_d = nc.declare_dram_parameter("x", [DIM, N], BF16, isOutput=False)
    # columns: [wq|wk | wk|wq | wv] (wq pre-scaled by head_dim**-0.5)
    wqkvT_d = nc.declare_dram_parameter("wqkvT", [DIM, 320], BF16,
                                        isOutput=False)
    woT_d = nc.declare_dram_parameter("woT", [HID, DIM], BF16, isOutput=False)
    b_d = nc.declare_dram_parameter("bout", [DIM], F32, isOutput=False)
    y_d = nc.declare_dram_parameter("y", [DIM, BLK], F32, isOutput=True)

    with tile.TileContext(nc) as tc, ExitStack() as ctx:
        const = ctx.enter_context(tc.tile_pool(name="const", bufs=1))
        sbA = ctx.enter_context(tc.tile_pool(name="sbA", bufs=1))
        pexp = ctx.enter_context(tc.tile_pool(name="pexp", bufs=6))
        psml = ctx.enter_context(tc.tile_pool(name="psml", bufs=3))
        dram = ctx.enter_context(tc.tile_pool(name="dram", bufs=1,
                                              space="DRAM"))
        psO = ctx.enter_context(tc.tile_pool(name="psO", bufs=2,
                                             space="PSUM"))

        # ---- constants / persistent SBUF ----
        wqkvT_sb = const.tile([128, 2, 320], BF16)
        nc.sync.dma_start(wqkvT_sb[:],
                          wqkvT_d.rearrange("(c p) m -> p c m", p=128))
        # final-projection weights loaded up front (off the critical tail)
        woT_sb = const.tile([128, 4, DIM], BF16)
        nc.sync.dma_start(woT_sb[:],
                          woT_d.rearrange("(c p) m -> p c m", p=128))
        b_sb = const.tile([128, 2], F32)
        nc.sync.dma_start(b_sb[:], b_d.rearrange("(m p) -> p m", p=128))

        nbias_sb = const.tile([128, 1], F32)
        nc.vector.memset(nbias_sb[:], -C_SHIFT)
        ones16_sb = const.tile([128, HD], F16)
        nc.vector.memset(ones16_sb[:], 1.0)

        x_sb = sbA.tile([128, 2, N], BF16)
        qk_sb = sbA.tile([128, N], BF16)   # partitions 0:64 = q, 64:128 = k
        kq_sb = sbA.tile([128, N], BF16)   # partitions 0:64 = k, 64:128 = q
        # v fp8 pair tiles: [p, pair, ktile, dim]; col 64 = ones so the
        # denominator accumulates in psum row 64; cols padded to 80 for the
        # DoubleRow ldweights 16B stride rule.
        vv_sb = sbA.tile([128, NG, 2, 80], FP8)
        nc.vector.memset(vv_sb[:], 0.0)
        nc.vector.memset(vv_sb[:, :, :, 64:65], 1.0)
        rhs_sb = sbA.tile([128, 4, BLK], BF16)

        a2a_in = dram.tile([NB, HD, BLK], BF16)
        a2a_out = dram.tile([NB, HD, BLK], BF16)
        warm_in = dram.tile([128, 4], F32)
        warm_out = dram.tile([128, 4], F32)
        warmz_sb = const.tile([128, 4], F32)
        nc.vector.memset(warmz_sb[:], 0.0)
        nc.sync.dma_start(warm_in[:], warmz_sb[:])
        # tiny warm-up collective: absorbs CC init cost under the preamble
        nc.gpsimd.collective_compute(
            "AllReduce", mybir.AluOpType.add,
            replica_groups=[list(range(N_CORES))],
            ins=[warm_in.opt()], outs=[warm_out.opt()])

        pending = []  # [countdown, fn] emitted in order once countdown <= 0

        def schedule(fn, delay):
            pending.append([delay, fn])

        def tick():
            for it in pending:
                it[0] -= 1
            for it in [it for it in pending if it[0] <= 0]:
                pending.remove(it)
                it[1]()

        def drain():
            while pending:
                pending.pop(0)[1]()

        def emit_stage_a(b, psA):
            bs = slice(b * BLK, (b + 1) * BLK)
            for c in range(2):
                nc.sync.dma_start(x_sb[:, c, bs],
                                  x_d[c * 128:(c + 1) * 128, bs])
            ps_qk = psA.tile([128, BLK], F32, tag="pa", name=f"psqk_{b}")
            for c in range(2):
                nc.tensor.matmul(ps_qk[:], wqkvT_sb[:, c, 0:128],
                                 x_sb[:, c, bs],
                                 start=(c == 0), stop=(c == 1))
            nc.vector.tensor_copy(qk_sb[:, bs], ps_qk[:])
            ps_kq = psA.tile([128, BLK], F32, tag="pa", name=f"pskq_{b}")
            for c in range(2):
                nc.tensor.matmul(ps_kq[:], wqkvT_sb[:, c, 128:256],
                                 x_sb[:, c, bs],
                                 start=(c == 0), stop=(c == 1))
            nc.vector.tensor_copy(kq_sb[:, bs], ps_kq[:])
            for t in range(4):
                nt = b * 4 + t
                g, kt = nt // 2, nt % 2
                ps_v = psA.tile([128, HD], F32, tag="pa", name=f"psv_{nt}")
                for c in range(2):
                    nc.tensor.matmul(
                        ps_v[:],
                        x_sb[:, c, nt * 128:(nt + 1) * 128],
                        wqkvT_sb[:, c, 256:320],
                        start=(c == 0), stop=(c == 1))
                nc.vector.tensor_copy(vv_sb[:, g, kt, 0:64], ps_v[:])

        def emit_pair(i, g, ps_out, psB, assign):
            isl = slice(i * BLK, (i + 1) * BLK)
            j0, j1 = 2 * g, 2 * g + 1
            psg = psB.tile([128, 2, BLK], F32, tag="psg",
                           name=f"psg_{i}_{g}")
            nc.tensor.matmul(psg[:, 0, :],
                             kq_sb[0:64, j0 * 128:(j0 + 1) * 128],
                             qk_sb[0:64, isl],
                             start=True, stop=True, tile_position=(0, 0))
            nc.tensor.matmul(psg[:, 1, :],
                             qk_sb[64:128, j1 * 128:(j1 + 1) * 128],
                             kq_sb[64:128, isl],
                             start=True, stop=True, tile_position=(64, 0))
            pe = pexp.tile([128, 2, BLK], FP8, tag="pe", name=f"pe_{i}_{g}")
            if assign[g % NG] == 'A':
                nc.scalar.activation(pe[:], psg[:], EXP, bias=nbias_sb[:])
            else:
                # saturating affine+u8 convert on DVE (hardware clamps
                # negative f32->u8 to 0; only CoreSim models a wrap)
                nc.vector.tensor_scalar(pe[:].bitcast(U8), psg[:],
                                        SCHR_SCALE, SCHR_BIAS,
                                        mybir.AluOpType.mult,
                                        mybir.AluOpType.add)

            def mk_outp():
                nc.tensor.matmul(ps_out[0:65, :], vv_sb[:, g, :, 0:65],
                                 pe[:], start=(g == 0), stop=(g == NG - 1),
                                 perf_mode=DR)
            schedule(mk_outp, 2)
            tick()

        def emit_norm(i, ps_out):
            oall = psml.tile([128, BLK], F32, tag="oall", name=f"oall_{i}")
            r16_sb = psml.tile([128, BLK], F16, tag="r16", name=f"r16_{i}")
            # raw dram tensors: DMA views of pool-tile rings mislower
            srow_d = nc.dram_tensor(f"srow_{i}", [BLK], F32, kind="Internal")
            rrow_d = nc.dram_tensor(f"rrow_{i}", [BLK], F32, kind="Internal")

            def mk_norm_a():
                nc.vector.tensor_copy(oall[0:65, :], ps_out[0:65, :])
                # lane-parallel reciprocal: bounce the s row through DRAM
                # repartitioned [128, 4] (a serial [1,512] recip is 3.3us)
                s128 = psml.tile([128, 4], F32, tag="s128",
                                 name=f"s128_{i}")
                r128 = psml.tile([128, 4], F32, tag="r128",
                                 name=f"r128_{i}")
                nc.sync.dma_start(srow_d.ap(), oall[64:65, :])
                nc.sync.dma_start(
                    s128[:], srow_d.ap().rearrange("(p f) -> p f", p=128))
                nc.vector.reciprocal(r128[:], s128[:])
                nc.sync.dma_start(
                    rrow_d.ap().rearrange("(p f) -> p f", p=128), r128[:])
                # casting DMA (gpsimd-only) brings 1/s back as one fp16 row
                nc.gpsimd.dma_start(
                    r16_sb[64:65, :],
                    rrow_d.ap().rearrange("(o n) -> o n", o=1))
            schedule(mk_norm_a, 2)

            def mk_norm_b():
                outn = psml.tile([HD, BLK], BF16, tag="outn",
                                 name=f"outn_{i}")
                # ps_out rows 0:64 are free once oall holds the copy: write
                # the fp16 K=1 broadcast of 1/s right back into them
                nc.tensor.matmul(ps_out[0:HD, :], ones16_sb[64:65, 0:HD],
                                 r16_sb[64:65, :], start=True, stop=True)
                nc.vector.tensor_mul(outn[:], oall[0:HD, :],
                                     ps_out[0:HD, :])
                nc.sync.dma_start(a2a_in[i], outn[:])
            schedule(mk_norm_b, 5)

        # ---- stage A interleaved with i-block 0 ----
        ps_out0 = psO.tile([128, BLK], F32, tag="psout", name="psout_0")
        with tc.tile_pool(name="psA", bufs=2, space="PSUM") as psA_pool, \
                tc.tile_pool(name="psB2", bufs=2, space="PSUM") as psB2:
            for b in range(NB):
                emit_stage_a(b, psA_pool)
                emit_pair(0, 2 * b, ps_out0, psB2, ASSIGN_BLK0)
                emit_pair(0, 2 * b + 1, ps_out0, psB2, ASSIGN_BLK0)
            emit_norm(0, ps_out0)

        # ---- i-blocks 1..7 ----
        with tc.tile_pool(name="psB3", bufs=3, space="PSUM") as psB3:
            for i in range(1, NB):
                ps_out = psO.tile([128, BLK], F32, tag="psout",
                                  name=f"psout_{i}")
                for g in range(NG):
                    emit_pair(i, g, ps_out, psB3, ASSIGN_MAIN)
                emit_norm(i, ps_out)
            drain()

            # ---- stage C: AllToAll over token blocks + output projection --
            nc.gpsimd.collective_compute(
                "AllToAll", mybir.AluOpType.bypass,
                replica_groups=[list(range(N_CORES))],
                ins=[a2a_in.opt()], outs=[a2a_out.opt()])

            a2a_r = a2a_out.rearrange("(c a) d t -> (a d) c t", c=4, a=2)
            ps_yt = psB3.tile([128, 2, BLK], F32, tag="psg", name="psy")
            for c in range(4):
                nc.gpsimd.dma_start(rhs_sb[:, c, :], a2a_r[:, c, :])
                for m in range(2):
                    nc.tensor.matmul(ps_yt[:, m, :],
                                     woT_sb[:, c, m * 128:(m + 1) * 128],
                                     rhs_sb[:, c, :],
                                     start=(c == 0), stop=(c == 3))
            for m in range(2):
                y_sb = psml.tile([128, BLK], F32, tag="ysb", name=f"ysb_{m}")
                nc.vector.tensor_scalar_add(y_sb[:], ps_yt[:, m, :],
                                            b_sb[:, m:m + 1])
                nc.sync.dma_start(y_d[m * 128:(m + 1) * 128, :], y_sb[:])

    nc.compile()
    return nc


def _make_in_maps(x, w_qkv, w_out, b_out):
    x2 = np.ascontiguousarray(
        np.asarray(x, np.float32).reshape(DIM, N)).astype(ml_dtypes.bfloat16)
    w_qkv = np.asarray(w_qkv, np.float32)
    scale = HD ** -0.5
    woT = np.ascontiguousarray(np.asarray(w_out, np.float32).T).astype(
        ml_dtypes.bfloat16)
    b = np.ascontiguousarray(np.asarray(b_out, np.float32).reshape(DIM))
    in_maps = []
    for h in range(N_CORES):
        wq = w_qkv[h * HD:(h + 1) * HD] * scale
        wk = w_qkv[HID + h * HD:HID + (h + 1) * HD]
        wv = w_qkv[2 * HID + h * HD:2 * HID + (h + 1) * HD]
        wqkvT = np.ascontiguousarray(
            np.concatenate([wq.T, wk.T, wk.T, wq.T, wv.T], axis=1),
            np.float32).astype(ml_dtypes.bfloat16)
        in_maps.append({"x": x2, "wqkvT": wqkvT, "woT": woT, "bout": b})
    return in_maps


def _assemble(results):
    y = np.concatenate([results[h]["y"] for h in range(N_CORES)], axis=1)
    return np.ascontiguousarray(y.reshape(1, DIM, 64, 64).astype(np.float32))


def kernel(x, w_qkv, w_out, b_out):
    nc = build_program()
    in_maps = _make_in_maps(x, w_qkv, w_out, b_out)
    res = run_bass_kernel_spmd(nc, in_maps, list(range(N_CORES)))
    return _assemble(res.results)


def run_traced(x, w_qkv, w_out, b_out, trace_cores=None):
    """Test-harness entry: also returns BassKernelResults with exec_time_ns."""
    nc = build_program()
    in_maps = _make_in_maps(x, w_qkv, w_out, b_out)
    res = run_bass_kernel_spmd(nc, in_maps, list(range(N_CORES)), trace=True,
                               trace_cores=trace_cores)
    return _assemble(res.results), res


# revision 34
# speedup vs baseline: 1.2563x; 1.1250x over previous
"""Multi-head self-attention (8 heads, head_dim 64, n=4096, dim=256) on 8
Trainium2 NeuronCores.

Sharding: one attention head per core (tensor parallel on the heads axis of
to_qkv / to_out). Each core:
  A) computes the dual-layout projections qk = [q; k] and kq = [k; q]
     (each 128 x 4096 bf16, q/k on opposite partition halves) plus v in
     fp8e4 (pair layout for DoubleRow, with a ones column so the softmax
     denominator falls out of the PE matmul),
  B) streams the 4096x4096 attention for its head in j-tile PAIRS:
     sim = k^T q for two j-tiles lands in one 2-bank PSUM tile; the
     shifted exp (logits - 2 keeps exp < 240 = fp8e4 max) is produced in
     fp8 by one of TWO engines round-robin — the scalar engine (exact exp,
     fp8 out) or the vector engine via a Schraudolph-style uint8 bit trick
     (affine + saturating u8 convert, bitcast as fp8e4) — so exp never
     gates the PE and the PE stays continuously busy (full 2.4 GHz
     p-state).  The attention-weighted value sum runs as one fp8 DoubleRow
     matmul per pair (2 j-tiles per ~256-cycle pass) accumulated in PSUM.
  C) normalizes by the fused row sum (copy + DVE reciprocal + a K=1 fp16
     matmul that replicates 1/s across partitions, written back into the
     just-freed PSUM bank), then an AllToAll gives every core the full
     512-channel hidden state for its own 512-token slice; the final
     projection + bias runs per core on that slice.
The host only reshapes/slices/casts inputs per core and concatenates the 8
disjoint token slices of the output.
"""

import os
import sys
from contextlib import ExitStack

for _p in ("/opt/trn_rl_repo",):
    if os.path.isdir(_p) and _p not in sys.path:
        sys.path.append(_p)

import ml_dtypes
import numpy as np

import concourse.bass as bass
import concourse.mybir as mybir
import concourse.tile as tile
from concourse import bacc
from concourse.bass_utils import run_bass_kernel_spmd

HEADS = 8
HD = 64           # head dim
DIM = 256         # model dim
N = 4096          # tokens (64*64)
HID = HEADS * HD  # 512
NB = 8            # token blocks
BLK = N // NB     # 512
NJ = N // 128     # 32 j-tiles of 128
NG = NJ // 2      # 16 j-tile pairs per i-block
N_CORES = 8

F32 = mybir.dt.float32
F16 = mybir.dt.float16
BF16 = mybir.dt.bfloat16
FP8 = mybir.dt.float8e4
U8 = mybir.dt.uint8
EXP = mybir.ActivationFunctionType.Exp
DR = mybir.MatmulPerfMode.DoubleRow

LOG2E = 1.4426950408889634
C_SHIFT = 2.0                                    # logit shift: exp < 240
SCHR_SCALE = float(8 * LOG2E)                    # fp8e4 Schraudolph slope
SCHR_BIAS = float(56.0 - C_SHIFT * 8 * LOG2E - 0.5)  # bias 7<<3, magic -0.5

# exp-engine round robin per pair: A=scalar(exact fp8), D=vector (schr-u8)
ASSIGN_MAIN = "ADAADADAADADAADA"   # 9 A, 7 D  (blocks 1..7)
ASSIGN_BLK0 = "ADADADADADADADAA"   # 9 A, 7 D   (stage A loads the engines)
if os.environ.get("KERNEL_ALL_SCHR", "0") == "1":
    ASSIGN_MAIN = ASSIGN_BLK0 = "D" * 16
if os.environ.get("KERNEL_ALL_ACT", "0") == "1":
    ASSIGN_MAIN = ASSIGN_BLK0 = "A" * 16


def build_program():
    nc = bacc.Bacc("TRN2", target_bir_lowering=False, debug=False,
                   num_devices=N_CORES)
    x_d = nc.declare_dram_parameter("x", [DIM, N], BF16, isOutput=False)
    # columns: [wq|wk | wk|wq | wv] (wq pre-scaled by head_dim**-0.5)
    wqkvT_d = nc.declare_dram_parameter("wqkvT", [DIM, 320], BF16,
                                        isOutput=False)
    woT_d = nc.declare_dram_parameter("woT", [HID, DIM], BF16, isOutput=False)
    b_d = nc.declare_dram_parameter("bout", [DIM], F32, isOutput=False)
    y_d = nc.declare_dram_parameter("y", [DIM, BLK], F32, isOutput=True)

    with tile.TileContext(nc) as tc, ExitStack() as ctx:
        const = ctx.enter_context(tc.tile_pool(name="const", bufs=1))
        sbA = ctx.enter_context(tc.tile_pool(name="sbA", bufs=1))
        pexp = ctx.enter_context(tc.tile_pool(name="pexp", bufs=8))
        psml = ctx.enter_context(tc.tile_pool(name="psml", bufs=3))
        dram = ctx.enter_context(tc.tile_pool(name="dram", bufs=1,
                                              space="DRAM"))
        psO = ctx.enter_context(tc.tile_pool(name="psO", bufs=2,
                                             space="PSUM"))

        # ---- constants / persistent SBUF ----
        wqkvT_sb = const.tile([128, 2, 320], BF16)
        nc.sync.dma_start(wqkvT_sb[:],
                          wqkvT_d.rearrange("(c p) m -> p c m", p=128))
        # final-projection weights loaded up front (off the critical tail)
        woT_sb = const.tile([128, 4, DIM], BF16)
        nc.sync.dma_start(woT_sb[:],
                          woT_d.rearrange("(c p) m -> p c m", p=128))
        b_sb = const.tile([128, 2], F32)
        nc.sync.dma_start(b_sb[:], b_d.rearrange("(m p) -> p m", p=128))

        nbias_sb = const.tile([128, 1], F32)
        nc.vector.memset(nbias_sb[:], -C_SHIFT)
        ones16_sb = const.tile([128, HD], F16)
        nc.vector.memset(ones16_sb[:], 1.0)

        x_sb = sbA.tile([128, 2, N], BF16)
        qk_sb = sbA.tile([128, N], BF16)   # partitions 0:64 = q, 64:128 = k
        kq_sb = sbA.tile([128, N], BF16)   # partitions 0:64 = k, 64:128 = q
        # v fp8 pair tiles: [p, pair, ktile, dim]; col 64 = ones so the
        # denominator accumulates in psum row 64; cols padded to 80 for the
        # DoubleRow ldweights 16B stride rule.
        vv_sb = sbA.tile([128, NG, 2, 80], FP8)
        nc.vector.memset(vv_sb[:], 0.0)
        nc.vector.memset(vv_sb[:, :, :, 64:65], 1.0)
        rhs_sb = sbA.tile([128, 4, BLK], BF16)

        a2a_in = dram.tile([NB, HD, BLK], BF16)
        a2a_out = dram.tile([NB, HD, BLK], BF16)
        warm_in = dram.tile([128, 4], F32)
        warm_out = dram.tile([128, 4], F32)
        warmz_sb = const.tile([128, 4], F32)
        nc.vector.memset(warmz_sb[:], 0.0)
        nc.sync.dma_start(warm_in[:], warmz_sb[:])
        # tiny warm-up collective: absorbs CC init cost under the preamble
        nc.gpsimd.collective_compute(
            "AllReduce", mybir.AluOpType.add,
            replica_groups=[list(range(N_CORES))],
            ins=[warm_in.opt()], outs=[warm_out.opt()])

        pending = []  # [countdown, fn] emitted in order once countdown <= 0

        def schedule(fn, delay):
            pending.append([delay, fn])

        def tick():
            for it in pending:
                it[0] -= 1
            for it in [it for it in pending if it[0] <= 0]:
                pending.remove(it)
                it[1]()

        def drain():
            while pending:
                pending.pop(0)[1]()

        def emit_stage_a(b, psA):
            bs = slice(b * BLK, (b + 1) * BLK)
            for c in range(2):
                nc.sync.dma_start(x_sb[:, c, bs],
                                  x_d[c * 128:(c + 1) * 128, bs])
            ps_qk = psA.tile([128, BLK], F32, tag="pa", name=f"psqk_{b}")
            for c in range(2):
                nc.tensor.matmul(ps_qk[:], wqkvT_sb[:, c, 0:128],
                                 x_sb[:, c, bs],
                                 start=(c == 0), stop=(c == 1))
            nc.vector.tensor_copy(qk_sb[:, bs], ps_qk[:])
            ps_kq = psA.tile([128, BLK], F32, tag="pa", name=f"pskq_{b}")
            for c in range(2):
                nc.tensor.matmul(ps_kq[:], wqkvT_sb[:, c, 128:256],
                                 x_sb[:, c, bs],
                                 start=(c == 0), stop=(c == 1))
            nc.vector.tensor_copy(kq_sb[:, bs], ps_kq[:])
            for t in range(4):
                nt = b * 4 + t
                g, kt = nt // 2, nt % 2
                ps_v = psA.tile([128, HD], F32, tag="pa", name=f"psv_{nt}")
                for c in range(2):
                    nc.tensor.matmul(
                        ps_v[:],
                        x_sb[:, c, nt * 128:(nt + 1) * 128],
                        wqkvT_sb[:, c, 256:320],
                        start=(c == 0), stop=(c == 1))
                nc.vector.tensor_copy(vv_sb[:, g, kt, 0:64], ps_v[:])

        def emit_pair(i, g, ps_out, psB, assign):
            isl = slice(i * BLK, (i + 1) * BLK)
            j0, j1 = 2 * g, 2 * g + 1
            psg = psB.tile([128, 2, BLK], F32, tag="psg",
                           name=f"psg_{i}_{g}")
            nc.tensor.matmul(psg[:, 0, :],
                             kq_sb[0:64, j0 * 128:(j0 + 1) * 128],
                             qk_sb[0:64, isl],
                             start=True, stop=True, tile_position=(0, 0))
            nc.tensor.matmul(psg[:, 1, :],
                             qk_sb[64:128, j1 * 128:(j1 + 1) * 128],
                             kq_sb[64:128, isl],
                             start=True, stop=True, tile_position=(64, 0))
            pe = pexp.tile([128, 2, BLK], FP8, tag="pe", name=f"pe_{i}_{g}")
            if assign[g % NG] == 'A':
                nc.scalar.activation(pe[:], psg[:], EXP, bias=nbias_sb[:])
            else:
                # saturating affine+u8 convert on DVE (hardware clamps
                # negative f32->u8 to 0; only CoreSim models a wrap)
                nc.vector.tensor_scalar(pe[:].bitcast(U8), psg[:],
                                        SCHR_SCALE, SCHR_BIAS,
                                        mybir.AluOpType.mult,
                                        mybir.AluOpType.add)

            def mk_outp():
                nc.tensor.matmul(ps_out[0:65, :], vv_sb[:, g, :, 0:65],
                                 pe[:], start=(g == 0), stop=(g == NG - 1),
                                 perf_mode=DR)
            schedule(mk_outp, 3)
            tick()

        def emit_norm(i, ps_out):
            oall = psml.tile([128, BLK], F32, tag="oall", name=f"oall_{i}")
            r16_sb = psml.tile([128, BLK], F16, tag="r16", name=f"r16_{i}")
            # raw dram tensors: DMA views of pool-tile rings mislower
            srow_d = nc.dram_tensor(f"srow_{i}", [BLK], F32, kind="Internal")
            rrow_d = nc.dram_tensor(f"rrow_{i}", [BLK], F32, kind="Internal")

            def mk_norm_a():
                nc.vector.tensor_copy(oall[0:65, :], ps_out[0:65, :])
                # lane-parallel reciprocal: bounce the s row through DRAM
                # repartitioned [128, 4] (a serial [1,512] recip is 3.3us)
                s128 = psml.tile([128, 4], F32, tag="s128",
                                 name=f"s128_{i}")
                r128 = psml.tile([128, 4], F32, tag="r128",
                                 name=f"r128_{i}")
                nc.sync.dma_start(srow_d.ap(), oall[64:65, :])
                nc.sync.dma_start(
                    s128[:], srow_d.ap().rearrange("(p f) -> p f", p=128))
                nc.vector.reciprocal(r128[:], s128[:])
                nc.sync.dma_start(
                    rrow_d.ap().rearrange("(p f) -> p f", p=128), r128[:])
                # casting DMA (gpsimd-only) brings 1/s back as one fp16 row
                nc.gpsimd.dma_start(
                    r16_sb[64:65, :],
                    rrow_d.ap().rearrange("(o n) -> o n", o=1))
            schedule(mk_norm_a, 2)

            def mk_norm_b():
                outn = psml.tile([HD, BLK], BF16, tag="outn",
                                 name=f"outn_{i}")
                # ps_out rows 0:64 are free once oall holds the copy: write
                # the fp16 K=1 broadcast of 1/s right back into them
                nc.tensor.matmul(ps_out[0:HD, :], ones16_sb[64:65, 0:HD],
                                 r16_sb[64:65, :], start=True, stop=True)
                nc.vector.tensor_mul(outn[:], oall[0:HD, :],
                                     ps_out[0:HD, :])
                nc.sync.dma_start(a2a_in[i], outn[:])
            schedule(mk_norm_b, 5)

        # ---- stage A interleaved with i-block 0 ----
        ps_out0 = psO.tile([128, BLK], F32, tag="psout", name="psout_0")
        with tc.tile_pool(name="psA", bufs=2, space="PSUM") as psA_pool, \
                tc.tile_pool(name="psB2", bufs=2, space="PSUM") as psB2:
            for b in range(NB):
                emit_stage_a(b, psA_pool)
                emit_pair(0, 2 * b, ps_out0, psB2, ASSIGN_BLK0)
                emit_pair(0, 2 * b + 1, ps_out0, psB2, ASSIGN_BLK0)
            emit_norm(0, ps_out0)

        # ---- i-blocks 1..7 ----
        with tc.tile_pool(name="psB3", bufs=3, space="PSUM") as psB3:
            for i in range(1, NB):
                ps_out = psO.tile([128, BLK], F32, tag="psout",
                                  name=f"psout_{i}")
                for g in range(NG):
                    emit_pair(i, g, ps_out, psB3, ASSIGN_MAIN)
                emit_norm(i, ps_out)
            drain()

            # ---- stage C: AllToAll over token blocks + output projection --
            nc.gpsimd.collective_compute(
                "AllToAll", mybir.AluOpType.bypass,
                replica_groups=[list(range(N_CORES))],
                ins=[a2a_in.opt()], outs=[a2a_out.opt()])

            a2a_r = a2a_out.rearrange("(c a) d t -> (a d) c t", c=4, a=2)
            ps_yt = psB3.tile([128, 2, BLK], F32, tag="psg", name="psy")
            for c in range(4):
                nc.gpsimd.dma_start(rhs_sb[:, c, :], a2a_r[:, c, :])
                for m in range(2):
                    nc.tensor.matmul(ps_yt[:, m, :],
                                     woT_sb[:, c, m * 128:(m + 1) * 128],
                                     rhs_sb[:, c, :],
                                     start=(c == 0), stop=(c == 3))
            for m in range(2):
                y_sb = psml.tile([128, BLK], F32, tag="ysb", name=f"ysb_{m}")
                nc.vector.tensor_scalar_add(y_sb[:], ps_yt[:, m, :],
                                            b_sb[:, m:m + 1])
                nc.sync.dma_start(y_d[m * 128:(m + 1) * 128, :], y_sb[:])

    nc.compile()
    return nc


def _make_in_maps(x, w_qkv, w_out, b_out):
    x2 = np.ascontiguousarray(
        np.asarray(x, np.float32).reshape(DIM, N)).astype(ml_dtypes.bfloat16)
    w_qkv = np.asarray(w_qkv, np.float32)
    scale = HD ** -0.5
    woT = np.ascontiguousarray(np.asarray(w_out, np.float32).T).astype(
        ml_dtypes.bfloat16)
    b = np.ascontiguousarray(np.asarray(b_out, np.float32).reshape(DIM))
    in_maps = []
    for h in range(N_CORES):
        wq = w_qkv[h * HD:(h + 1) * HD] * scale
        wk = w_qkv[HID + h * HD:HID + (h + 1) * HD]
        wv = w_qkv[2 * HID + h * HD:2 * HID + (h + 1) * HD]
        wqkvT = np.ascontiguousarray(
            np.concatenate([wq.T, wk.T, wk.T, wq.T, wv.T], axis=1),
            np.float32).astype(ml_dtypes.bfloat16)
        in_maps.append({"x": x2, "wqkvT": wqkvT, "woT": woT, "bout": b})
    return in_maps


def _assemble(results):
    y = np.concatenate([results[h]["y"] for h in range(N_CORES)], axis=1)
    return np.ascontiguousarray(y.reshape(1, DIM, 64, 64).astype(np.float32))


def kernel(x, w_qkv, w_out, b_out):
    nc = build_program()
    in_maps = _make_in_maps(x, w_qkv, w_out, b_out)
    res = run_bass_kernel_spmd(nc, in_maps, list(range(N_CORES)))
    return _assemble(res.results)


def run_traced(x, w_qkv, w_out, b_out, trace_cores=None):
    """Test-harness entry: also returns BassKernelResults with exec_time_ns."""
    nc = build_program()
    in_maps = _make_in_maps(x, w_qkv, w_out, b_out)
    res = run_bass_kernel_spmd(nc, in_maps, list(range(N_CORES)), trace=True,
                               trace_cores=trace_cores)
    return _assemble(res.results), res
